# revision 33
# baseline (speedup 1.0000x reference)
"""Trainium2 Bass kernel for nn_AttentionBlock (GroupNorm + attention block),
data-parallel over batch across 8 NeuronCores.

Reference computation (per batch element b, C=512, N=H*W=1024, heads=8, hd=64):
  xn   = GroupNorm32(x) * gamma + beta
  qkv  = w_qkv @ xn + b_qkv        (1x1 conv == matmul over channels)
  attn = softmax(q^T k / sqrt(hd)) ; ha = attn @ v ; out = x + w_proj @ ha + b_proj
Sharding: batch B=8 -> one batch element per core. No collectives.

Final design (v13). ScalarE runs only the softmax exp (one ACT table
set, preloaded by a dummy exp during the input DMAs); everything else is
arranged so the 64 x ~1.1us exp stream is the pacing engine:
  - Host side: weights pre-transposed and pre-cast (bf16), x pre-cast to
    bf16 (halves the input DMA; costs ~6e-4 rel err via the residual), all
    DMAs contiguous and spread over the sync/scalar/gpsimd DGE queues.
  - k bias dropped entirely: softmax is shift-invariant along the key axis,
    so only q keeps its bias (applied at PSUM eviction on DVE).
  - GroupNorm: bn_stats/bn_aggr per half-chunk on DVE; group reduce and
    re-broadcast via tiny TensorE matmuls with +-1/16 selector matrices;
    rsqrt(var+eps) from an int-shift 0x5F3759DF seed + one Newton step
    (standard DVE ops - custom DVE ops are broken on this runtime, and
    Ln/Sqrt would each load another ACT table set). The xn affine runs
    cc0-1 on DVE and cc2-3 on the otherwise-idle prologue ScalarE
    (activation accepts per-partition scale/bias APs).
  - Heads processed in PAIRS (head 2p at partitions 0-63, 2p+1 at 64-127).
    Each S_T m-chunk writes ONE psum tile per nt (A in cols 0:512, B in
    512:1024): both K=64 matmuls become ready together, so the PE co-issues
    them into disjoint row groups (measured 4ns apart), and a single
    FD=1024 exp covers both heads.
  - P_T = exp(S_T/8 - ln 16) written as fp8e4m3 (the 1/16 keeps P inside
    e4m3 range and cancels in the softmax normalization; no max
    subtraction needed since |scores/8| <~ 8).
  - AV: ha_u[80, n] = [v_T | ones16]^T @ P_T in fp8 DoubleRow mode over
    m-chunk PAIRS (each matmul contracts 256 rows); psum rows 64-79 hold
    the denominator Z.
  - 1/Z: RECIPROCAL_APPROX_FAST's algorithm decomposed into standard DVE
    ops (bitwise_not seed, Chebyshev pair c0/c1, one fused NR step):
      w1 = (Z*(~Z)*c0 - c1) * (~Z);  1/Z ~= -c0*w1   (~0.2% rel err)
    with -c0/16 folded into the zb broadcast selector; all Z scratch lives
    on partitions 64-79 to stay aligned with the PSUM Z rows.
  - zb = broadcast(1/Z) to 64 partitions via a K=16 selector matmul, PSUM
    eviction copies on ScalarE (DVE copies concurrent with the exp stream
    cause a global slowdown on this part), ha = ha_u * zb on DVE.
  - Steady-state steps: S_T/exp of pair p overlapped with the previous
    pair's AV/normalization and the next pair's q/k as PE filler; step 3
    trails its own AV so the epilogue starts immediately.
  - proj: out = (psum + b_proj) + x fused in one DVE scalar_tensor_tensor,
    evicted and DMAed per 512-column half; the first two output tiles
    defer their last cc chunk so proj matmuls cover the final pair's
    normalization chain.
Measured: 163.5us HW exec (baseline 213.2us), rel err 2.7e-3 (tol 2e-2).
"""

import os

import numpy as np
import ml_dtypes

import concourse.bass as bass
import concourse.bacc as bacc
import concourse.mybir as mybir
import concourse.tile as tile
from concourse.bass_utils import run_bass_kernel_spmd

F32 = mybir.dt.float32
FP8 = mybir.dt.float8e4
BF16 = mybir.dt.bfloat16
I32 = mybir.dt.int32
AF = mybir.ActivationFunctionType
ALU = mybir.AluOpType

B = 8
C = 512
N = 1024          # H*W = 32*32
H = 8             # num heads
HD = 64           # head dim
G = 32            # groups
GS = C // G       # channels per group = 16
CCH = 4           # channel chunks of 128
NT = 2            # n tiles of 512
MT = 8            # m tiles of 128
EPS = 1e-5
P = 128
NCORES = 8
W80 = HD + 16     # v columns + 16 ones columns per head

RC0 = -0.23549792          # Chebyshev recip seed scale
RC1 = 2.0017324
RSQRT_MAGIC = 0x5F3759DF

_CACHE = {}


def build_nc():
    nc = bacc.Bacc(
        "TRN2", target_bir_lowering=False, debug=False, num_devices=NCORES
    )

    x_d = nc.declare_dram_parameter("x", [C, N], BF16, isOutput=False)
    wqk_d = nc.declare_dram_parameter("w_qkT", [P, CCH * 2 * C], BF16, isOutput=False)
    wv_d = nc.declare_dram_parameter("w_vT", [P, CCH * C], BF16, isOutput=False)
    wvb_d = nc.declare_dram_parameter("w_vb", [1, C], BF16, isOutput=False)
    wp_d = nc.declare_dram_parameter("w_pT", [P, CCH * C], BF16, isOutput=False)
    pf_d = nc.declare_dram_parameter("pf32", [P, 16], F32, isOutput=False)
    pb_d = nc.declare_dram_parameter("pbf16", [P, 200], BF16, isOutput=False)
    out_d = nc.declare_dram_parameter("out", [C, N], F32, isOutput=True)

    with tile.TileContext(nc) as tc:
        with (
            tc.tile_pool(name="singles", bufs=1) as singles,
            tc.tile_pool(name="outbuf", bufs=2) as outbuf,
            tc.tile_pool(name="ps", bufs=2, space="PSUM") as ps_pool,
            tc.tile_pool(name="ps_av", bufs=2, space="PSUM") as ps_av_pool,
        ):
            # ---------------- static tiles ----------------
            x_sb = singles.tile([P, CCH, N], BF16)
            xn_sb = singles.tile([P, CCH, N], BF16)
            wqk_sb = singles.tile([P, CCH, 2 * C], BF16)
            wv_sb = singles.tile([P, CCH, C], BF16)
            wvb_sb = singles.tile([1, C], BF16)
            wp_sb = singles.tile([P, CCH, C], BF16)
            pf_sb = singles.tile([P, 16], F32)    # bq(4) bp(4) gamma(4) beta(4)
            pb_sb = singles.tile([P, 200], BF16)  # gsel(8) gselT(128) zsel(64)

            qk_sb = singles.tile([P, 8, N], BF16)       # ot 0-3: q, 4-7: k
            vT_sb = singles.tile([P, MT, H * W80], FP8)
            pT_t = [
                singles.tile([P, MT, NT, N], FP8, name=f"pT{i}")
                for i in range(2)
            ]
            ha_sb = singles.tile([P, CCH, N], BF16)
            ones_row = singles.tile([1, P], BF16)

            # GroupNorm scratch
            bns_sb = singles.tile([P, CCH, 2, 6], F32)  # bn_stats per half-chunk
            aggr_sb = singles.tile([P, CCH, 2], F32)    # (mean, var) per channel
            msq_sb = singles.tile([P, CCH], F32)
            ex2c_sb = singles.tile([P, CCH], F32)
            s12_bf = singles.tile([P, 8], BF16)
            sq_scr = singles.tile([P, N], F32)
            mu_rs = singles.tile([8, 8], F32)           # cols 0-3 mu, 4-7 rs
            mu_rs_bf = singles.tile([8, 8], BF16)
            ex2_sb = singles.tile([8, CCH], F32)
            tmp8 = singles.tile([8, CCH], F32)
            var_sb = singles.tile([8, CCH], F32)        # then var+eps
            rsq_i = singles.tile([8, CCH], I32)         # int seed scratch
            rsq_r = singles.tile([8, CCH], F32)         # rsqrt iterate
            rsq_t = singles.tile([8, CCH], F32)
            rsq_u = singles.tile([8, CCH], F32)
            s0_sb = singles.tile([P, CCH], F32)
            sbias_sb = singles.tile([P, CCH], F32)
            tmp128 = singles.tile([P, CCH], F32)

            # softmax 1/Z scratch: rows 64-79 only (aligned with PSUM Z rows);
            # col range [0, N) = head A of the pair, [N, 2N) = head B.
            zw_sb = singles.tile([P, 2 * N], F32)       # ~Z seed (bits)
            zt_sb = singles.tile([P, 2 * N], F32)       # Z * y0
            zi_sb = singles.tile([P, 2 * N], BF16)      # w1 (recip * -1/c0)
            zb_t = [singles.tile([HD, N], F32, name=f"zb{i}") for i in range(4)]
            ebias_sb = singles.tile([P, 1], F32)        # -ln(16) exp bias

            # ---------------- input DMAs (3 queues) ----------------
            x_v = x_d.ap().rearrange("(cc p) n -> p cc n", p=P)
            for cc in range(CCH):
                eng = nc.sync if cc < 2 else nc.gpsimd
                eng.dma_start(x_sb[:, cc, :], x_v[:, cc, :])
            nc.scalar.dma_start(
                wqk_sb[:], wqk_d.ap().rearrange("p (cc o) -> p cc o", cc=CCH)
            )
            nc.scalar.dma_start(
                wv_sb[:], wv_d.ap().rearrange("p (cc o) -> p cc o", cc=CCH)
            )
            nc.scalar.dma_start(wvb_sb[:], wvb_d.ap())
            nc.scalar.dma_start(
                wp_sb[:], wp_d.ap().rearrange("p (cc o) -> p cc o", cc=CCH)
            )
            nc.gpsimd.dma_start(pf_sb[:], pf_d.ap())
            nc.gpsimd.dma_start(pb_sb[:], pb_d.ap())

            bq = pf_sb[:, 0:4]
            bp = pf_sb[:, 4:8]
            gm = pf_sb[:, 8:12]
            bt = pf_sb[:, 12:16]
            gsel = pb_sb[:, 0:8]
            gselT = pb_sb[0:8, 8:136]
            zsel = pb_sb[64:80, 136:200]   # [16, 64] = -c0/16

            nc.vector.memset(ones_row[:], 1.0)
            nc.vector.memset(ebias_sb[:], -2.772588722239781)
            nc.vector.memset(
                vT_sb[:].rearrange("p mt (h d) -> p mt h d", h=H)[:, :, :, HD:W80],
                1.0,
            )
            # Preload the exp ACT table set while the input DMAs run.
            nc.scalar.activation(sq_scr[0:1, 0:P], ones_row[:], AF.Exp)

            # ---------------- GroupNorm stats ----------------
            for cc in range(CCH):
                for hf in range(2):
                    nc.vector.bn_stats(
                        bns_sb[:, cc, hf, :],
                        x_sb[:, cc, hf * 512 : (hf + 1) * 512],
                    )
                nc.vector.bn_aggr(aggr_sb[:, cc, :], bns_sb[:, cc, :, :])
            nc.vector.tensor_mul(msq_sb[:], aggr_sb[:, :, 0], aggr_sb[:, :, 0])
            nc.vector.tensor_add(ex2c_sb[:], msq_sb[:], aggr_sb[:, :, 1])
            nc.vector.tensor_copy(s12_bf[:, 0:4], aggr_sb[:, :, 0])
            nc.vector.tensor_copy(s12_bf[:, 4:8], ex2c_sb[:])
            ps_st = ps_pool.tile([P, N], F32, tag="ps")
            nc.tensor.matmul(
                ps_st[0:8, 0:8], gsel, s12_bf[:], start=True, stop=True
            )
            inv_cnt = 1.0 / GS
            nc.vector.tensor_scalar_mul(mu_rs[:, 0:4], ps_st[0:8, 0:4], inv_cnt)
            nc.vector.tensor_scalar_mul(ex2_sb[:], ps_st[0:8, 4:8], inv_cnt)
            nc.vector.tensor_mul(tmp8[:], mu_rs[:, 0:4], mu_rs[:, 0:4])
            nc.vector.tensor_sub(var_sb[:], ex2_sb[:], tmp8[:])
            nc.vector.tensor_scalar_add(var_sb[:], var_sb[:], EPS)
            # rsqrt(var+eps): int seed MAGIC - (bits >> 1), then 2 Newton steps
            nc.vector.tensor_scalar(
                out=rsq_i[:], in0=var_sb[:].bitcast(I32),
                scalar1=1, scalar2=None, op0=ALU.arith_shift_right,
            )
            nc.vector.tensor_scalar_sub(rsq_i[:], rsq_i[:], RSQRT_MAGIC)
            nc.vector.tensor_scalar(            # ~x (then +1 below: -x = ~x+1)
                out=rsq_i[:], in0=rsq_i[:],
                scalar1=0, scalar2=None, op0=ALU.bitwise_not,
            )
            nc.vector.tensor_scalar_add(rsq_r[:].bitcast(I32), rsq_i[:], 1)
            for _ in range(1):
                nc.vector.tensor_mul(rsq_t[:], rsq_r[:], rsq_r[:])
                nc.vector.scalar_tensor_tensor(
                    out=rsq_u[:], in0=rsq_t[:], scalar=-0.5, in1=var_sb[:],
                    op0=ALU.mult, op1=ALU.mult,
                )
                nc.vector.scalar_tensor_tensor(
                    out=rsq_r[:], in0=rsq_u[:], scalar=1.5, in1=rsq_r[:],
                    op0=ALU.add, op1=ALU.mult,
                )
            nc.vector.tensor_copy(mu_rs[:, 4:8], rsq_r[:])
            nc.vector.tensor_copy(mu_rs_bf[:], mu_rs[:])
            ps_bc = ps_pool.tile([P, N], F32, tag="ps")
            nc.tensor.matmul(
                ps_bc[0:P, 0:8], gselT, mu_rs_bf[:], start=True, stop=True
            )
            nc.vector.tensor_mul(s0_sb[:], ps_bc[0:P, 4:8], gm)
            nc.vector.tensor_mul(tmp128[:], ps_bc[0:P, 0:4], s0_sb[:])
            nc.vector.tensor_sub(sbias_sb[:], bt, tmp128[:])
            for cc in range(CCH):
                if cc < 2:
                    nc.vector.tensor_scalar(
                        out=xn_sb[:, cc, :],
                        in0=x_sb[:, cc, :],
                        scalar1=s0_sb[:, cc : cc + 1],
                        scalar2=sbias_sb[:, cc : cc + 1],
                        op0=ALU.mult,
                        op1=ALU.add,
                    )
                else:
                    nc.scalar.activation(
                        xn_sb[:, cc, :], x_sb[:, cc, :], AF.Identity,
                        bias=sbias_sb[:, cc : cc + 1],
                        scale=s0_sb[:, cc : cc + 1],
                    )

            # ---------------- emission helpers ----------------
            def qk_ot(ot):
                """q (ot<4) or k (ot>=4) output tile: 8 matmuls + eviction."""
                ps_qk = ps_pool.tile([P, N], F32, tag="ps", name=f"qk{ot}")
                for cc in range(CCH):
                    for nt in range(NT):
                        nc.tensor.matmul(
                            ps_qk[:, nt * 512 : (nt + 1) * 512],
                            wqk_sb[:, cc, ot * P : (ot + 1) * P],
                            xn_sb[:, cc, nt * 512 : (nt + 1) * 512],
                            start=(cc == 0),
                            stop=(cc == CCH - 1),
                        )
                if ot < 4:  # q: add bias on eviction
                    nc.vector.tensor_scalar_add(
                        qk_sb[:, ot, :], ps_qk[:], bq[:, ot : ot + 1]
                    )
                else:       # k: bias cancels in softmax; plain copy
                    nc.vector.tensor_copy(qk_sb[:, ot, :], ps_qk[:])

            pending_vcopy = []

            def v_mt(mt):
                """v_T chunk for rows [128*mt, 128*mt+128): 5 matmuls. The
                ACT eviction copy is deferred until after the next S_T
                chunk's exps so it never head-of-line-blocks the exp FIFO."""
                ps_v = ps_pool.tile([P, N], F32, tag="ps", name=f"v{mt}")
                for cc in range(CCH):
                    nc.tensor.matmul(
                        ps_v[:, 0:C],
                        xn_sb[:, cc, mt * P : (mt + 1) * P],
                        wv_sb[:, cc, :],
                        start=(cc == 0),
                        stop=False,
                    )
                nc.tensor.matmul(
                    ps_v[:, 0:C], ones_row[:], wvb_sb[:], start=False, stop=True
                )
                pending_vcopy.append((mt, ps_v))

            def flush_vcopy():
                for m, ps_v in pending_vcopy:
                    nc.vector.tensor_copy(
                        vT_sb[:, m, :]
                        .rearrange("p (h d) -> p h d", h=H)[:, :, 0:HD],
                        ps_v[:, 0:C].rearrange("p (h d) -> p h d", h=H),
                    )
                pending_vcopy.clear()

            def st_pair_mt(pr, mt):
                """S_T + exp for heads (2pr, 2pr+1), m-chunk mt. Both heads'
                K=64 matmuls write ONE psum tile (A cols 0:512, B 512:1024)
                so they become ready together and the PE co-issues them into
                disjoint row groups (~2x). One exp covers both heads."""
                ot = pr
                pt = pT_t[pr % 2]
                scale = float(HD) ** -0.5
                for nt in range(NT):
                    sl = slice(nt * 512, (nt + 1) * 512)
                    ps = ps_pool.tile([P, N], F32, tag="ps", name=f"s{pr}_{mt}_{nt}")
                    nc.tensor.matmul(
                        ps[:, 512:1024],
                        qk_sb[HD:P, 4 + ot, mt * P : (mt + 1) * P],
                        qk_sb[HD:P, ot, sl],
                        start=True, stop=True,
                    )
                    nc.tensor.matmul(
                        ps[:, 0:512],
                        qk_sb[0:HD, 4 + ot, mt * P : (mt + 1) * P],
                        qk_sb[0:HD, ot, sl],
                        start=True, stop=True,
                    )
                    nc.scalar.activation(
                        pt[:, mt, nt, :], ps[:], AF.Exp, scale=scale,
                        bias=ebias_sb[:],
                    )

            def av_mt(pr, mp, ps_avA, ps_avB):
                """AV accumulation for m-chunk PAIR (2mp, 2mp+1), head pair
                pr: 4 fp8 DoubleRow matmuls, each contracting 256 rows."""
                hA, hB = 2 * pr, 2 * pr + 1
                pt = pT_t[pr % 2]
                for (h, half, ps_av) in ((hA, 0, ps_avA), (hB, 512, ps_avB)):
                    for nt in range(NT):
                        sl = slice(nt * 512, (nt + 1) * 512)
                        nc.tensor.matmul(
                            ps_av[0:W80, sl],
                            vT_sb[:, 2 * mp : 2 * mp + 2,
                                  h * W80 : (h + 1) * W80],
                            pt[:, 2 * mp : 2 * mp + 2, nt, half : half + 512],
                            start=(mp == 0),
                            stop=(mp == MT // 2 - 1),
                            perf_mode=mybir.MatmulPerfMode.DoubleRow,
                        )

            def norm_pair_dve(pr, ps_avA, ps_avB):
                """w1 = (Z*~Z*c0 - c1)*~Z per head (fast recip minus the final
                scale, folded into zsel). Standard DVE ops only."""
                for (ps_av, off) in ((ps_avA, 0), (ps_avB, N)):
                    zr = ps_av[HD:W80, :]
                    sl = slice(off, off + N)
                    nc.vector.tensor_scalar(
                        out=zw_sb[HD:W80, sl].bitcast(I32),
                        in0=zr.bitcast(I32),
                        scalar1=0, scalar2=None, op0=ALU.bitwise_not,
                    )
                    nc.vector.scalar_tensor_tensor(   # zt = (~Z * c0) * Z
                        out=zt_sb[HD:W80, sl],
                        in0=zw_sb[HD:W80, sl],
                        scalar=RC0,
                        in1=zr,
                        op0=ALU.mult, op1=ALU.mult,
                    )
                    nc.vector.scalar_tensor_tensor(   # w1 = (zt - c1) * ~Z
                        out=zi_sb[HD:W80, sl],
                        in0=zt_sb[HD:W80, sl],
                        scalar=RC1,
                        in1=zw_sb[HD:W80, sl],
                        op0=ALU.subtract, op1=ALU.mult,
                    )

            def zb_pair(pr):
                """zb = broadcast(1/Z) to 64 partitions (selector carries the
                -c0/16 scale)."""
                zbA = zb_t[(pr % 2) * 2]
                zbB = zb_t[(pr % 2) * 2 + 1]
                ps_zbA = ps_pool.tile([P, N], F32, tag="ps", name=f"zbA{pr}")
                ps_zbB = ps_pool.tile([P, N], F32, tag="ps", name=f"zbB{pr}")
                for nt in range(NT):
                    sl = slice(nt * 512, (nt + 1) * 512)
                    nc.tensor.matmul(
                        ps_zbA[0:HD, sl], zsel, zi_sb[HD:W80, sl],
                        start=True, stop=True,
                    )
                    nc.tensor.matmul(
                        ps_zbB[0:HD, sl], zsel,
                        zi_sb[HD:W80, N + sl.start : N + sl.stop],
                        start=True, stop=True,
                    )
                nc.scalar.copy(zbA[:], ps_zbA[0:HD, :])
                nc.scalar.copy(zbB[:], ps_zbB[0:HD, :])

            def zb_one(pr, side):
                """zb broadcast for one head (A: side=0, B: side=1) using a
                single psum slot, freed immediately by its ACT copy."""
                zb = zb_t[(pr % 2) * 2 + side]
                ps_zb = ps_pool.tile([P, N], F32, tag="ps", name=f"zb{pr}_{side}")
                off = side * N
                for nt in range(NT):
                    sl = slice(nt * 512, (nt + 1) * 512)
                    nc.tensor.matmul(
                        ps_zb[0:HD, sl], zsel,
                        zi_sb[HD:W80, off + sl.start : off + sl.stop],
                        start=True, stop=True,
                    )
                nc.scalar.copy(zb[:], ps_zb[0:HD, :])

            def ha_pair(pr, ps_avA, ps_avB):
                ot = pr
                zbA = zb_t[(pr % 2) * 2]
                zbB = zb_t[(pr % 2) * 2 + 1]
                nc.vector.tensor_mul(ha_sb[0:HD, ot, :], ps_avA[0:HD, :], zbA[:])
                nc.vector.tensor_mul(ha_sb[HD:P, ot, :], ps_avB[0:HD, :], zbB[:])

            def proj_ot_mms(ot, ps_p, ccs):
                for cc in ccs:
                    for nt in range(NT):
                        nc.tensor.matmul(
                            ps_p[:, nt * 512 : (nt + 1) * 512],
                            wp_sb[:, cc, ot * P : (ot + 1) * P],
                            ha_sb[:, cc, nt * 512 : (nt + 1) * 512],
                            start=(cc == 0),
                            stop=(cc == CCH - 1),
                        )

            # ---------------- prologue: q0/k0 ----------------
            qk_ot(0)
            qk_ot(4)

            # ---------------- head-pair pipeline ----------------
            # Step 0: S_T/exp of pair 0, with v chunks and q1/k1 as PE filler.
            for mt in range(MT):
                st_pair_mt(0, mt)
                flush_vcopy()
                v_mt(mt)
                if mt == 2:
                    qk_ot(1)
                if mt == 5:
                    qk_ot(5)

            # Steps 1-3: S_T/exp of pair pr overlapped with AV of pair pr-1
            # (AV one chunk ahead so Z completes while S_T still has work).
            av3A = av3B = None
            for pr in range(1, 4):
                avA = ps_av_pool.tile([P, N], F32, tag="av", name=f"avA{pr-1}")
                avB = ps_av_pool.tile([P, N], F32, tag="av", name=f"avB{pr-1}")
                st_pair_mt(pr, 0)
                st_pair_mt(pr, 1)
                flush_vcopy()
                av_mt(pr - 1, 0, avA, avB)
                av_mt(pr - 1, 1, avA, avB)
                st_pair_mt(pr, 2)
                av_mt(pr - 1, 2, avA, avB)
                st_pair_mt(pr, 3)
                av_mt(pr - 1, 3, avA, avB)
                norm_pair_dve(pr - 1, avA, avB)
                st_pair_mt(pr, 4)
                if pr < 3:
                    qk_ot(pr + 1)
                if pr == 3:
                    av3A = ps_av_pool.tile([P, N], F32, tag="av", name="avA3")
                    av3B = ps_av_pool.tile([P, N], F32, tag="av", name="avB3")
                    av_mt(3, 0, av3A, av3B)
                st_pair_mt(pr, 5)
                zb_pair(pr - 1)
                if pr == 3:
                    av_mt(3, 1, av3A, av3B)
                st_pair_mt(pr, 6)
                if pr < 3:
                    qk_ot(pr + 5)
                if pr == 3:
                    av_mt(3, 2, av3A, av3B)
                ha_pair(pr - 1, avA, avB)
                st_pair_mt(pr, 7)
                if pr == 3:
                    av_mt(3, 3, av3A, av3B)

            # Final: normalization of pair 3 straight after its trailing AV.
            norm_pair_dve(3, av3A, av3B)
            zb_pair(3)
            ha_pair(3, av3A, av3B)

            # ---------------- proj + bias + residual ----------------
            out_v = out_d.ap().rearrange("(ot p) n -> p ot n", p=P)

            def proj_evict(ot, ps_p):
                out_t = outbuf.tile([P, N], F32, tag="out")
                for nt in range(NT):
                    sl = slice(nt * 512, (nt + 1) * 512)
                    nc.vector.scalar_tensor_tensor(
                        out=out_t[:, sl],
                        in0=ps_p[:, sl],
                        scalar=bp[:, ot : ot + 1],
                        in1=x_sb[:, ot, sl],
                        op0=ALU.add,
                        op1=ALU.add,
                    )
                    nc.sync.dma_start(out_v[:, ot, sl], out_t[:, sl])

            ps_p0 = ps_pool.tile([P, N], F32, tag="ps", name="proj0")
            proj_ot_mms(0, ps_p0, range(CCH - 1))
            ps_p1 = ps_pool.tile([P, N], F32, tag="ps", name="proj1")
            proj_ot_mms(1, ps_p1, range(CCH - 1))
            proj_ot_mms(0, ps_p0, [CCH - 1])
            proj_evict(0, ps_p0)
            proj_ot_mms(1, ps_p1, [CCH - 1])
            proj_evict(1, ps_p1)
            for ot in range(2, CCH):
                ps_p = ps_pool.tile([P, N], F32, tag="ps", name=f"proj{ot}")
                proj_ot_mms(ot, ps_p, range(CCH))
                proj_evict(ot, ps_p)

    nc.compile()
    return nc


def make_in_maps(x, gn_gamma, gn_beta, w_qkv, b_qkv, w_proj, b_proj):
    f32 = np.float32
    bf16 = ml_dtypes.bfloat16
    w_qkv = np.asarray(w_qkv, dtype=f32)
    b_qkv = np.asarray(b_qkv, dtype=f32)

    def chunked_T(w):
        # [O, C_in] -> transposed [C_in, O] -> SBUF layout [p, cc, O] packed
        # as [P, cc*O] with in-channel c = cc*128 + p.
        wt = np.ascontiguousarray(w.T, dtype=f32)  # [C_in, O]
        o = wt.shape[1]
        return np.ascontiguousarray(
            wt.reshape(CCH, P, o).transpose(1, 0, 2).reshape(P, CCH * o)
        ).astype(bf16)

    def perch(v):
        return np.asarray(v, dtype=f32).reshape(CCH, P).T  # [P, CCH]

    pf32 = np.concatenate(
        [
            perch(b_qkv[:C]),          # q bias
            perch(np.asarray(b_proj, dtype=f32)),
            perch(np.asarray(gn_gamma, dtype=f32)),
            perch(np.asarray(gn_beta, dtype=f32)),
        ],
        axis=1,
    ).astype(f32)

    # pbf16: gsel [P, 0:8], gselT [rows 0-7, 8:136], zsel [rows 64-79, 136:200]
    gsel = np.zeros((P, 8), f32)
    for p in range(P):
        gsel[p, p // GS] = 1.0
    pbf16 = np.zeros((P, 200), f32)
    pbf16[:, 0:8] = gsel
    pbf16[0:8, 8:136] = gsel.T
    pbf16[64:80, 136:200] = -RC0 / 16.0
    pbf16 = np.ascontiguousarray(pbf16).astype(bf16)

    shared = {
        "w_qkT": chunked_T(w_qkv[: 2 * C]),
        "w_vT": chunked_T(w_qkv[2 * C :]),
        "w_vb": np.ascontiguousarray(b_qkv[2 * C :][None, :]).astype(bf16),
        "w_pT": chunked_T(np.asarray(w_proj, dtype=f32)),
        "pf32": pf32,
        "pbf16": pbf16,
    }
    in_maps = []
    for b in range(B):
        m = dict(shared)
        m["x"] = np.ascontiguousarray(
            np.asarray(x[b], dtype=f32).reshape(C, N)
        ).astype(bf16)
        in_maps.append(m)
    return in_maps


def kernel(x, gn_gamma, gn_beta, w_qkv, b_qkv, w_proj, b_proj):
    if "nc" not in _CACHE:
        _CACHE["nc"] = build_nc()
    nc = _CACHE["nc"]
    in_maps = make_in_maps(x, gn_gamma, gn_beta, w_qkv, b_qkv, w_proj, b_proj)
    trace = bool(os.environ.get("KERNEL_TRACE"))
    res = run_bass_kernel_spmd(
        nc, in_maps, core_ids=list(range(NCORES)), trace=trace
    )
    _CACHE["last_result"] = res
    out = np.stack([np.asarray(res.results[i]["out"]) for i in range(NCORES)])
    return out.reshape(B, C, 32, 32).astype(np.float32)
